# revision 1
# baseline (speedup 1.0000x reference)
"""Fused multi-head attention + LayerNorm kernel for 8 Trainium2 NeuronCores.

Problem (hardcoded): B=4, S=2048, DIM=1024, H=16, HD=64; out = LayerNorm(
softmax(q W_q^T (k W_k^T)^T / sqrt(HD)) (v W_v^T) W_o^T + b_o) per reference.

Sharding: core c -> batch b = c//2, head-group g = c%2 (8 heads / 512 features).
The two cores of a pair exchange normalized attention outputs (AllGather over
pairs) so each finalizes half of the tokens.

Per-core dataflow (feature-major transposed layouts everywhere):
  1. q/k projections in bf16 -> qhT/khT [64(hd), 8(head), 2048(tok)];
     v projection in f32r -> vh_aug [128(j), 16(jt), 583] packed per head as
     64 values + a ones column (next head's data acts as harmless padding up
     to the 128-wide stationary).
  2. Attention per (head, j-tile): scoresT = khT^T qhT (PE, K=64 bf16),
     ET = exp(scale * scoresT) on ScalarE (psum -> sbuf f32r),
     outT_aug += vh_aug^T ET on PE (f32r); row 64 accumulates the softmax
     denominator thanks to the ones column.
  3. Rowsum reciprocal (DVE) -> gpsimd partition-broadcast -> normalize on
     eviction into out_normT (bf16).
  4. AllGather the cross token-half within the pair.
  5. Output projection (bf16) + bias + LayerNorm (bn_stats) -> y half.

Host tricks: each core's q tokens are permuted so "my half" is always columns
0:1024; Wo rows are rotated per core so the [mine, peer] chunk order matches.
"""
import sys

sys.path.insert(0, "/opt/trn_rl_repo")

import numpy as np
import ml_dtypes

B, S, DIM, H, HD = 4, 2048, 1024, 16, 64
NCORES = 8
NH = 8             # heads per core
FL = NH * HD       # 512 local features
EPS = 1e-5
SCALE = HD ** -0.5
P = 128
JT = S // P        # 16
IC = S // 512      # 4
TH = S // 2        # 1024 tokens finalized per core
DC = DIM // P      # 8 contraction chunks
HW = HD + 1        # 65: head block width in vh_aug
VW = NH * HW             # 520 (stationary windows spill into next block)

_cache = {}


def _build():
    import os
    STAGE = int(os.environ.get("STAGE", "4"))
    import concourse.bass as bass
    import concourse.bacc as bacc
    import concourse.tile as tile
    from concourse import mybir
    f32 = mybir.dt.float32
    f32r = mybir.dt.float32r
    bf16 = mybir.dt.bfloat16
    i32 = mybir.dt.int32
    EXPF = mybir.ActivationFunctionType.Exp
    SQRTF = mybir.ActivationFunctionType.Sqrt
    ALU = mybir.AluOpType
    ds = bass.ds

    nc = bacc.Bacc("TRN2", target_bir_lowering=False, debug=False,
                   num_devices=NCORES)

    xqT_d = nc.dram_tensor("xqT", [DIM, S], bf16, kind="ExternalInput")
    xkT_d = nc.dram_tensor("xkT", [DIM, S], bf16, kind="ExternalInput")
    xvT_d = nc.dram_tensor("xvT", [DIM, S], f32r, kind="ExternalInput")
    wqT_d = nc.dram_tensor("wqT", [DIM, FL], bf16, kind="ExternalInput")
    wkT_d = nc.dram_tensor("wkT", [DIM, FL], bf16, kind="ExternalInput")
    wvT_d = nc.dram_tensor("wvT", [DIM, FL], f32r, kind="ExternalInput")
    woT_d = nc.dram_tensor("woT", [DIM, DIM], bf16, kind="ExternalInput")
    bq_d = nc.dram_tensor("bq", [FL], f32, kind="ExternalInput")
    bk_d = nc.dram_tensor("bk", [FL], f32, kind="ExternalInput")
    bv_d = nc.dram_tensor("bv", [FL], f32, kind="ExternalInput")
    bo_d = nc.dram_tensor("bo", [DIM], f32, kind="ExternalInput")
    gamma_d = nc.dram_tensor("gamma", [DIM], f32, kind="ExternalInput")
    beta_d = nc.dram_tensor("beta", [DIM], f32, kind="ExternalInput")
    pidx_d = nc.dram_tensor("pidx", [1, 1], i32, kind="ExternalInput")
    y_d = nc.dram_tensor("y", [TH, DIM], f32, kind="ExternalOutput")

    PAIRS = [[0, 1], [2, 3], [4, 5], [6, 7]]

    def bcast_ap(ap, parts):
        return bass.AP(tensor=ap.tensor, offset=ap.offset,
                       ap=[[0, parts]] + list(ap.ap))

    with tile.TileContext(nc) as tc:
        import contextlib
        with contextlib.ExitStack() as ctx:
            persist = ctx.enter_context(tc.tile_pool(name="persist", bufs=1))
            ws = ctx.enter_context(tc.tile_pool(name="ws", bufs=1))
            xs = ctx.enter_context(tc.tile_pool(name="xs", bufs=3))
            et_pool = ctx.enter_context(tc.tile_pool(name="et", bufs=2))
            bc_pool = ctx.enter_context(tc.tile_pool(name="bc", bufs=2))
            ln_pool = ctx.enter_context(tc.tile_pool(name="ln", bufs=2))
            dram = ctx.enter_context(
                tc.tile_pool(name="dram", bufs=1, space="DRAM"))
            scp = ctx.enter_context(
                tc.tile_pool(name="scp", bufs=2, space="PSUM"))
            avp = ctx.enter_context(
                tc.tile_pool(name="avp", bufs=4, space="PSUM"))

            # ---------------- persistent state ----------------
            qhT = persist.tile([HD, NH, S], bf16)
            vh_aug = persist.tile([P, JT * VW + (P - HW)], f32r)
            out_normT = persist.tile([P, FL // P, S], bf16)
            rs64 = persist.tile([HD + 1, IC, 512], f32)   # row 64 stages sums

            # ones columns (rest of vh_aug holds data or harmless garbage;
            # garbage feeds only psum rows 65:127 which are never read)
            ones_f32 = persist.tile([P, P - HW], f32)
            nc.vector.memset(ones_f32, 1.0)
            vh_view = vh_aug[:, :JT * VW].rearrange("p (j w) -> p j w", w=VW)
            for h in range(NH):
                nc.scalar.copy(vh_view[:, :, h * HW + HD], ones_f32[:, :JT])
            # tail pad after the last head's window must be finite
            nc.scalar.copy(vh_aug[:, JT * VW:], ones_f32)

            bq_sb = persist.tile([P, FL // P], f32)
            bk_sb = persist.tile([P, FL // P], f32)
            nc.sync.dma_start(bq_sb, bq_d.rearrange("(a p) -> p a", p=P))
            nc.sync.dma_start(bk_sb, bk_d.rearrange("(a p) -> p a", p=P))
            bv_bc = persist.tile([P, FL], f32)
            nc.gpsimd.dma_start(bv_bc, bcast_ap(bv_d[:], P))
            bo_bc = persist.tile([P, DIM], f32)
            nc.gpsimd.dma_start(bo_bc, bcast_ap(bo_d[:], P))
            gamma_bc = persist.tile([P, DIM], f32)
            nc.gpsimd.dma_start(gamma_bc, bcast_ap(gamma_d[:], P))
            beta_bc = persist.tile([P, DIM], f32)
            nc.gpsimd.dma_start(beta_bc, bcast_ap(beta_d[:], P))
            eps_sb = persist.tile([P, 1], f32)
            nc.vector.memset(eps_sb, EPS)
            pidx_sb = persist.tile([1, 1], i32)
            nc.sync.dma_start(pidx_sb, pidx_d[:])

            # weights: wq slot reused by peer_T, wv slot reused by woT
            wq_sb = ws.tile([P, DC, FL], bf16, tag="wA")
            wk_sb = ws.tile([P, DC, FL], bf16, tag="wB")
            wv_sb = ws.tile([P, DC, FL], f32r, tag="wC")
            nc.sync.dma_start(wq_sb, wqT_d.rearrange("(a p) f -> p a f", p=P))
            nc.sync.dma_start(wk_sb, wkT_d.rearrange("(a p) f -> p a f", p=P))
            nc.sync.dma_start(wv_sb, wvT_d.rearrange("(a p) f -> p a f", p=P))

            # ---------------- q/k projections ----------------
            khT_dram = dram.tile([NH, HD, S], bf16)
            scope_stack = []
            def enter_scope(nm):
                while scope_stack:
                    n0, sid = scope_stack.pop()
                    nc.leave_named_scope(n0, sid, False)
                scope_stack.append((nm, nc.enter_named_scope(nm, False)[0]))
            enter_scope("proj_qk")
            for name, x_d, w_sb, b_sb, dstT in (
                ("q", xqT_d, wq_sb, bq_sb, qhT),
                ("k", xkT_d, wk_sb, bk_sb, None),
            ):
                for tci in range(IC):
                    xt = []
                    for dc in range(DC):
                        x_tile = xs.tile([P, 512], bf16, tag="xqk",
                                         name=f"x{name}_{tci}_{dc}")
                        nc.sync.dma_start(
                            x_tile,
                            x_d[dc * P:(dc + 1) * P, tci * 512:(tci + 1) * 512])
                        xt.append(x_tile)
                    for fc in range(FL // P):
                        ps = avp.tile([P, 512], f32, tag="ps512",
                                      name=f"ps_{name}_{tci}_{fc}")
                        for dc in range(DC):
                            nc.tensor.matmul(
                                ps, w_sb[:, dc, fc * P:(fc + 1) * P], xt[dc],
                                start=(dc == 0), stop=(dc == DC - 1))
                        for hf in range(2):
                            h = 2 * fc + hf
                            if dstT is not None:
                                nc.vector.tensor_scalar_add(
                                    dstT[:, h, tci * 512:(tci + 1) * 512],
                                    ps[hf * HD:(hf + 1) * HD],
                                    b_sb[hf * HD:(hf + 1) * HD, fc:fc + 1])
                            else:
                                kstg = xs.tile([HD, 512], bf16, tag="kstg",
                                               name=f"kstg_{tci}_{h}")
                                nc.vector.tensor_scalar_add(
                                    kstg, ps[hf * HD:(hf + 1) * HD],
                                    b_sb[hf * HD:(hf + 1) * HD, fc:fc + 1])
                                nc.sync.dma_start(
                                    khT_dram[h, :,
                                             tci * 512:(tci + 1) * 512],
                                    kstg)

            # ---------------- v projection ----------------
            enter_scope("proj_v")
            for jt in range(JT):
                ps = avp.tile([P, FL], f32, tag="ps512", name=f"ps_v_{jt}")
                for dc in range(DC):
                    xv_tile = xs.tile([P, P], f32r, tag="xv",
                                      name=f"xv_{jt}_{dc}")
                    nc.sync.dma_start(
                        xv_tile,
                        xvT_d[dc * P:(dc + 1) * P, jt * P:(jt + 1) * P])
                    nc.tensor.matmul(ps, xv_tile, wv_sb[:, dc, :],
                                     start=(dc == 0), stop=(dc == DC - 1))
                for h in range(NH):
                    nc.vector.scalar_tensor_tensor(
                        vh_aug[:, jt * VW + h * HW:jt * VW + h * HW + HD],
                        ps[:, h * HD:(h + 1) * HD], 0.0,
                        bv_bc[:, h * HD:(h + 1) * HD],
                        op0=ALU.add, op1=ALU.add)

            # ---------------- attention ----------------
            enter_scope("attn")
            for h in range(NH if STAGE >= 2 else 0):
                pav = [avp.tile([P, 512], f32, tag="ps512",
                                name=f"pav_{h}_{i}") for i in range(IC)]
                for jt in range(JT):
                    kst = xs.tile([HD, P], bf16, tag="kst", bufs=4,
                                  name=f"kst_{h}_{jt}")
                    nc.sync.dma_start(kst, khT_dram[h, :, jt * P:(jt + 1) * P])
                    for half in range(2):
                        psc = scp.tile([P, 1024], f32, tag="sc",
                                       name=f"sc_{h}_{jt}_{half}")
                        for i2 in range(2):
                            ic = half * 2 + i2
                            nc.tensor.matmul(
                                psc[:, i2 * 512:(i2 + 1) * 512],
                                kst,
                                qhT[:, h, ic * 512:(ic + 1) * 512],
                                start=True, stop=True)
                        et = et_pool.tile([P, 1024], f32r, tag="et",
                                          name=f"et_{h}_{jt}_{half}")
                        nc.scalar.activation(et, psc, EXPF, scale=SCALE)
                        for i2 in range(2):
                            ic = half * 2 + i2
                            nc.tensor.matmul(
                                pav[ic],
                                vh_aug[:, jt * VW + h * HW:
                                       jt * VW + h * HW + P],
                                et[:, i2 * 512:(i2 + 1) * 512],
                                start=(jt == 0), stop=(jt == JT - 1))
                # evict raw (rows 0:64 data, row 64 = denominator)
                for ic in range(IC):
                    nc.vector.tensor_copy(
                        out_normT[(h % 2) * HD:(h % 2) * HD + HD,
                                  h // 2, ic * 512:(ic + 1) * 512],
                        pav[ic][:HD])
                # rowsums -> partition 0 -> reciprocal -> broadcast -> scale
                for ic in range(IC):
                    nc.vector.tensor_copy(rs64[HD:HD + 1, ic, :],
                                          pav[ic][HD:HD + 1])
                    nc.sync.dma_start(rs64[0:1, ic, :], rs64[HD:HD + 1, ic, :])
                    rrec = bc_pool.tile([1, 2, 512], f32, tag="rrec",
                                        name=f"rrec_{h}_{ic}")
                    nc.vector.reciprocal_approx_accurate(
                        rrec[:, 0, :], rs64[0:1, ic, :], rrec[:, 1, :])
                    rbc = bc_pool.tile([P, 512], f32, tag="rbc",
                                       name=f"rbc_{h}_{ic}")
                    nc.gpsimd.partition_broadcast(rbc, rrec[:, 0, :])
                    hb = (h % 2) * HD
                    dst = out_normT[hb:hb + HD,
                                    h // 2, ic * 512:(ic + 1) * 512]
                    nc.gpsimd.tensor_mul(dst, dst, rbc[hb:hb + HD])

            # ---------------- pair exchange ----------------
            enter_scope("exch")
            cc_in = dram.tile([FL, TH], bf16)
            cc_out = dram.tile([2 * FL, TH], bf16)
            if STAGE >= 3:
                peer_T = ws.tile([P, FL // P, TH], bf16, tag="wA")
                for pi in range(FL // P):
                    nc.sync.dma_start(cc_in[pi * P:(pi + 1) * P, :],
                                      out_normT[:, pi, TH:S])
                nc.gpsimd.collective_compute(
                    "AllGather", ALU.bypass, replica_groups=PAIRS,
                    ins=[cc_in.opt()], outs=[cc_out.opt()])
                # peer = (slot0 + slot1) - mine, exact in f32, static reads
                for pi in range(FL // P):
                    ta = xs.tile([P, TH], bf16, tag="cca", bufs=2,
                                 name=f"cca_{pi}")
                    tb = xs.tile([P, TH], bf16, tag="ccb", bufs=2,
                                 name=f"ccb_{pi}")
                    nc.sync.dma_start(ta, cc_out[pi * P:(pi + 1) * P, :])
                    nc.sync.dma_start(tb, cc_out[FL + pi * P:
                                                 FL + (pi + 1) * P, :])
                    tmp = xs.tile([P, TH], f32, tag="cct", bufs=2,
                                  name=f"cct_{pi}")
                    nc.vector.tensor_add(tmp, ta, tb)
                    nc.vector.scalar_tensor_tensor(
                        peer_T[:, pi, :], tmp, 0.0, out_normT[:, pi, TH:S],
                        op0=ALU.add, op1=ALU.subtract)

            # ---------------- output projection + layernorm ----------------
            enter_scope("final")
            woT_sb = ws.tile([P, DC, DIM], bf16, tag="wC")
            nc.sync.dma_start(woT_sb, woT_d.rearrange("(a p) f -> p a f", p=P))
            for it in range(TH // P if STAGE >= 3 else 0):
                psf = [avp.tile([P, 512], f32, tag="ps512",
                                name=f"psf_{it}_{f}") for f in range(2)]
                for fcc in range(2):
                    for cc in range(DC):
                        if cc < 4:
                            stat = out_normT[:, cc, it * P:(it + 1) * P]
                        else:
                            stat = peer_T[:, cc - 4, it * P:(it + 1) * P]
                        nc.tensor.matmul(
                            psf[fcc], stat,
                            woT_sb[:, cc, fcc * 512:(fcc + 1) * 512],
                            start=(cc == 0), stop=(cc == DC - 1))
                xln = ln_pool.tile([P, DIM], f32, tag="xln", name=f"xln_{it}")
                for fcc in range(2):
                    nc.vector.scalar_tensor_tensor(
                        xln[:, fcc * 512:(fcc + 1) * 512], psf[fcc], 0.0,
                        bo_bc[:, fcc * 512:(fcc + 1) * 512],
                        op0=ALU.add, op1=ALU.add)
                stats = ln_pool.tile([P, 2, 6], f32, tag="st", name=f"st_{it}")
                for hf in range(2):
                    nc.vector.bn_stats(stats[:, hf, :],
                                       xln[:, hf * 512:(hf + 1) * 512])
                mv = ln_pool.tile([P, 2], f32, tag="mv", name=f"mv_{it}")
                nc.vector.bn_aggr(mv, stats)
                rstd = ln_pool.tile([P, 1], f32, tag="rstd", name=f"rstd_{it}")
                nc.scalar.activation(rstd, mv[:, 1:2], SQRTF, bias=eps_sb)
                nc.vector.reciprocal(rstd, rstd)
                nc.vector.scalar_tensor_tensor(
                    xln, xln, mv[:, 0:1], gamma_bc,
                    op0=ALU.subtract, op1=ALU.mult)
                nc.vector.scalar_tensor_tensor(
                    xln, xln, rstd, beta_bc, op0=ALU.mult, op1=ALU.add)
                nc.sync.dma_start(y_d[it * P:(it + 1) * P, :], xln)
            while scope_stack:
                n0, sid = scope_stack.pop()
                nc.leave_named_scope(n0, sid, False)

    nc.compile()
    return nc


def kernel(q, k, v, Wq, bq, Wk, bk, Wv, bv, Wo, bo, gamma, beta):
    from concourse.bass_utils import run_bass_kernel_spmd

    if "nc" not in _cache:
        _cache["nc"] = _build()
    nc = _cache["nc"]

    q = np.asarray(q, np.float32)
    k = np.asarray(k, np.float32)
    v = np.asarray(v, np.float32)
    Wq = np.asarray(Wq, np.float32); Wk = np.asarray(Wk, np.float32)
    Wv = np.asarray(Wv, np.float32); Wo = np.asarray(Wo, np.float32)
    bf = ml_dtypes.bfloat16
    WoT = Wo.T  # [c, f]
    in_maps = []
    for c in range(NCORES):
        b, g = c // 2, c % 2
        fsl = slice(g * FL, (g + 1) * FL)
        # my token half first (so "my half" is always columns 0:TH)
        xq = q[b].T if g == 0 else np.concatenate(
            [q[b].T[:, TH:], q[b].T[:, :TH]], axis=1)
        # Wo rows rotated: [my features, peer features]
        woT = np.concatenate([WoT[g * FL:(g + 1) * FL],
                              WoT[(1 - g) * FL:(2 - g) * FL]], axis=0)
        in_maps.append({
            "xqT": np.ascontiguousarray(xq).astype(bf),
            "xkT": np.ascontiguousarray(k[b].T).astype(bf),
            "xvT": np.ascontiguousarray(v[b].T).astype(np.float32),
            "wqT": np.ascontiguousarray(Wq[fsl, :].T).astype(bf),
            "wkT": np.ascontiguousarray(Wk[fsl, :].T).astype(bf),
            "wvT": np.ascontiguousarray(Wv[fsl, :].T).astype(np.float32),
            "woT": np.ascontiguousarray(woT).astype(bf),
            "bq": np.asarray(bq, np.float32)[fsl],
            "bk": np.asarray(bk, np.float32)[fsl],
            "bv": np.asarray(bv, np.float32)[fsl],
            "bo": np.asarray(bo, np.float32),
            "gamma": np.asarray(gamma, np.float32),
            "beta": np.asarray(beta, np.float32),
            "pidx": np.array([[1 - g]], np.int32),
        })
    res = run_bass_kernel_spmd(nc, in_maps, list(range(NCORES)),
                               trace=_cache.get("trace", False))
    _cache["last_res"] = res
    y = np.empty((B, S, DIM), np.float32)
    for c in range(NCORES):
        b, g = c // 2, c % 2
        y[b, g * TH:(g + 1) * TH, :] = res.results[c]["y"]
    return y



# revision 5
# speedup vs baseline: 1.9917x; 1.9917x over previous
"""Fused multi-head attention + LayerNorm kernel for 8 Trainium2 NeuronCores.

Problem (hardcoded): B=4, S=2048, DIM=1024, H=16, HD=64; out = LayerNorm(
softmax(q W_q^T (k W_k^T)^T / sqrt(HD)) (v W_v^T) W_o^T + b_o) per reference.

Sharding: core c -> batch b = c//2, head-group g = c%2 (8 heads / 512 features).
The two cores of a pair exchange normalized attention outputs (AllGather over
pairs) so each finalizes half of the tokens.

v2 design notes (vs the earlier baseline):
  - Everything bf16 except PSUM accumulation (f32r v-path removed; f32r moving
    ran at half rate cold and the extra bytes bought nothing).
  - khT stays resident in SBUF (no DRAM round trip between projection and
    attention).
  - Attention inner loop is software-pipelined: scores for jt+2 are issued
    before AV of jt so the PE never sits behind the ScalarE exp.
  - Denominator: ones-column trick -> PSUM row 64; evict does
    reciprocal (DVE) -> partition_broadcast (GpSimd) -> fused multiply on
    eviction (DVE) straight into out_normT.
  - Peer token-half (cols TH:S) is attended FIRST for all heads, the pair
    AllGather fires, and the own-half attention hides the collective +
    peer reconstruction.

Host tricks: each core's q tokens are permuted so "my half" is always columns
0:1024; Wo rows are rotated per core so the [mine, peer] chunk order matches.
"""
import sys

sys.path.insert(0, "/opt/trn_rl_repo")

import numpy as np
import ml_dtypes

B, S, DIM, H, HD = 4, 2048, 1024, 16, 64
NCORES = 8
NH = 8             # heads per core
FL = NH * HD       # 512 local features
EPS = 1e-5
SCALE = HD ** -0.5
P = 128
JT = S // P        # 16
IC = S // 512      # 4
TH = S // 2        # 1024 tokens finalized per core
DC = DIM // P      # 8 contraction chunks
HW = HD + 1        # 65: head block width in vh_aug
VW = NH * HW       # 520 (stationary windows spill into next block)

_cache = {}


def _build():
    import concourse.bass as bass
    import concourse.bacc as bacc
    import concourse.tile as tile
    from concourse import mybir
    f32 = mybir.dt.float32
    bf16 = mybir.dt.bfloat16
    EXPF = mybir.ActivationFunctionType.Exp
    IDENT = mybir.ActivationFunctionType.Identity
    SQRTF = mybir.ActivationFunctionType.Sqrt
    ALU = mybir.AluOpType

    nc = bacc.Bacc("TRN2", target_bir_lowering=False, debug=False,
                   num_devices=NCORES)

    xqT_d = nc.dram_tensor("xqT", [DIM, S], bf16, kind="ExternalInput")
    xkT_d = nc.dram_tensor("xkT", [DIM, S], bf16, kind="ExternalInput")
    xvT_d = nc.dram_tensor("xvT", [DIM, S], bf16, kind="ExternalInput")
    wqT_d = nc.dram_tensor("wqT", [DIM, FL], bf16, kind="ExternalInput")
    wkT_d = nc.dram_tensor("wkT", [DIM, FL], bf16, kind="ExternalInput")
    wvT_d = nc.dram_tensor("wvT", [DIM, FL], bf16, kind="ExternalInput")
    woT_d = nc.dram_tensor("woT", [DIM, DIM], bf16, kind="ExternalInput")
    bq_d = nc.dram_tensor("bq", [FL], f32, kind="ExternalInput")
    bk_d = nc.dram_tensor("bk", [FL], f32, kind="ExternalInput")
    bv_d = nc.dram_tensor("bv", [FL], f32, kind="ExternalInput")
    bo_d = nc.dram_tensor("bo", [DIM], f32, kind="ExternalInput")
    gamma_d = nc.dram_tensor("gamma", [DIM], f32, kind="ExternalInput")
    beta_d = nc.dram_tensor("beta", [DIM], f32, kind="ExternalInput")
    y_d = nc.dram_tensor("y", [TH, DIM], f32, kind="ExternalOutput")

    PAIRS = [[0, 1], [2, 3], [4, 5], [6, 7]]

    def bcast_ap(ap, parts):
        return bass.AP(tensor=ap.tensor, offset=ap.offset,
                       ap=[[0, parts]] + list(ap.ap))

    with tile.TileContext(nc) as tc:
        import contextlib
        with contextlib.ExitStack() as ctx:
            persist = ctx.enter_context(tc.tile_pool(name="persist", bufs=1))
            ws = ctx.enter_context(tc.tile_pool(name="ws", bufs=1))
            xs = ctx.enter_context(tc.tile_pool(name="xs", bufs=2))
            et_pool = ctx.enter_context(tc.tile_pool(name="et", bufs=3))
            bc_pool = ctx.enter_context(tc.tile_pool(name="bc", bufs=2))
            ln_pool = ctx.enter_context(tc.tile_pool(name="ln", bufs=2))
            dram = ctx.enter_context(
                tc.tile_pool(name="dram", bufs=1, space="DRAM"))
            scp = ctx.enter_context(
                tc.tile_pool(name="scp", bufs=2, space="PSUM"))
            avp = ctx.enter_context(
                tc.tile_pool(name="avp", bufs=4, space="PSUM"))

            # ---------------- persistent state ----------------
            qhT = persist.tile([HD, NH, S], bf16)
            khT = persist.tile([HD, NH, S], bf16)
            vh_aug = persist.tile([P, JT * VW + (P - HW)], bf16)
            out_normT = persist.tile([P, FL // P, S], bf16)

            # ones columns (rest of vh_aug holds data or harmless garbage;
            # garbage feeds only psum rows 65:127 which are never read)
            ones_bf = persist.tile([P, P - HW], bf16)
            nc.vector.memset(ones_bf, 1.0)
            vh_view = vh_aug[:, :JT * VW].rearrange("p (j w) -> p j w", w=VW)
            for h in range(NH):
                nc.scalar.copy(vh_view[:, :, h * HW + HD], ones_bf[:, :JT])
            # tail pad after the last head's window must be finite
            nc.scalar.copy(vh_aug[:, JT * VW:], ones_bf)

            bq_sb = persist.tile([P, FL // P], f32)
            bk_sb = persist.tile([P, FL // P], f32)
            nc.sync.dma_start(bq_sb, bq_d.rearrange("(a p) -> p a", p=P))
            nc.sync.dma_start(bk_sb, bk_d.rearrange("(a p) -> p a", p=P))
            bv_bc = persist.tile([P, FL], f32)
            nc.gpsimd.dma_start(bv_bc, bcast_ap(bv_d[:], P))
            bo_bc = persist.tile([P, DIM], f32)
            nc.gpsimd.dma_start(bo_bc, bcast_ap(bo_d[:], P))
            gamma_bc = persist.tile([P, DIM], f32)
            nc.gpsimd.dma_start(gamma_bc, bcast_ap(gamma_d[:], P))
            beta_bc = persist.tile([P, DIM], f32)
            nc.gpsimd.dma_start(beta_bc, bcast_ap(beta_d[:], P))
            eps_sb = persist.tile([P, 1], f32)
            nc.vector.memset(eps_sb, EPS)

            wq_sb = ws.tile([P, DC, FL], bf16, tag="wA")
            wk_sb = ws.tile([P, DC, FL], bf16, tag="wB")
            wv_sb = ws.tile([P, DC, FL], bf16, tag="wC")
            woT_sb = ws.tile([P, DC, DIM], bf16, tag="wO")
            nc.sync.dma_start(wq_sb, wqT_d.rearrange("(a p) f -> p a f", p=P))
            nc.sync.dma_start(wk_sb, wkT_d.rearrange("(a p) f -> p a f", p=P))
            nc.sync.dma_start(wv_sb, wvT_d.rearrange("(a p) f -> p a f", p=P))
            nc.sync.dma_start(woT_sb, woT_d.rearrange("(a p) f -> p a f", p=P))

            scope_stack = []

            def enter_scope(nm):
                while scope_stack:
                    n0, sid = scope_stack.pop()
                    nc.leave_named_scope(n0, sid, False)
                scope_stack.append((nm, nc.enter_named_scope(nm, False)[0]))

            # ---------------- q/k projections ----------------
            enter_scope("proj")
            for name, x_d, w_sb, b_sb, dstT in (
                ("q", xqT_d, wq_sb, bq_sb, qhT),
                ("k", xkT_d, wk_sb, bk_sb, khT),
            ):
                for tci in range(IC):
                    xt = xs.tile([P, DC, 512], bf16, tag="xqk",
                                 name=f"x{name}_{tci}")
                    for dc in range(DC):
                        nc.sync.dma_start(
                            xt[:, dc, :],
                            x_d[dc * P:(dc + 1) * P,
                                tci * 512:(tci + 1) * 512])
                    for fc in range(FL // P):
                        ps = avp.tile([P, 512], f32, tag="ps512",
                                      name=f"ps_{name}_{tci}_{fc}")
                        for dc in range(DC):
                            nc.tensor.matmul(
                                ps, w_sb[:, dc, fc * P:(fc + 1) * P],
                                xt[:, dc, :],
                                start=(dc == 0), stop=(dc == DC - 1))
                        for hf in range(2):
                            h = 2 * fc + hf
                            nc.scalar.activation(
                                dstT[:, h, tci * 512:(tci + 1) * 512],
                                ps[hf * HD:(hf + 1) * HD], IDENT,
                                bias=b_sb[hf * HD:(hf + 1) * HD, fc:fc + 1])

            # ---------------- v projection ----------------
            vh3 = vh_aug[:, :JT * VW].rearrange(
                "p (j h w) -> p j h w", h=NH, w=HW)
            bv3 = bv_bc.rearrange("p (h w) -> p h w", w=HD)
            for tci in range(IC):
                xvt = xs.tile([P, DC, 512], bf16, tag="xqk",
                              name=f"xv_{tci}")
                for dc in range(DC):
                    nc.sync.dma_start(
                        xvt[:, dc, :],
                        xvT_d[dc * P:(dc + 1) * P,
                              tci * 512:(tci + 1) * 512])
                for tj in range(4):
                    jt = tci * 4 + tj
                    ps = avp.tile([P, FL], f32, tag="ps512",
                                  name=f"ps_v_{jt}")
                    for dc in range(DC):
                        nc.tensor.matmul(
                            ps, xvt[:, dc, tj * P:(tj + 1) * P],
                            wv_sb[:, dc, :],
                            start=(dc == 0), stop=(dc == DC - 1))
                    nc.vector.scalar_tensor_tensor(
                        vh3[:, jt, :, :HD],
                        ps.rearrange("p (h w) -> p h w", w=HD), 0.0,
                        bv3, op0=ALU.add, op1=ALU.add)

            # ---------------- attention ----------------
            # half=1 (peer token half, cols TH:S) first so the pair exchange
            # overlaps the half=0 attention.
            cc_in = dram.tile([FL, TH], bf16)
            cc_sum = dram.tile([FL, TH], bf16)

            def attn_head(half, h):
                colb = half * TH
                pav = [avp.tile([P, 512], f32, tag="ps512",
                                name=f"pav_{half}_{h}_{i}") for i in range(2)]
                et_t = {}

                def S_step(jt):
                    psc = scp.tile([P, 1024], f32, tag="sc",
                                   name=f"sc_{half}_{h}_{jt}")
                    for i2 in range(2):
                        nc.tensor.matmul(
                            psc[:, i2 * 512:(i2 + 1) * 512],
                            khT[:, h, jt * P:(jt + 1) * P],
                            qhT[:, h, colb + i2 * 512:colb + (i2 + 1) * 512],
                            start=True, stop=True)
                    et = et_pool.tile([P, 1024], bf16, tag="et",
                                      name=f"et_{half}_{h}_{jt}")
                    nc.scalar.activation(et, psc, EXPF, scale=SCALE)
                    et_t[jt] = et

                def AV_step(jt):
                    et = et_t.pop(jt)
                    for i2 in range(2):
                        nc.tensor.matmul(
                            pav[i2],
                            vh_aug[:, jt * VW + h * HW:jt * VW + h * HW + P],
                            et[:, i2 * 512:(i2 + 1) * 512],
                            start=(jt == 0), stop=(jt == JT - 1))

                S_step(0)
                S_step(1)
                for jt in range(JT):
                    if jt + 2 < JT:
                        S_step(jt + 2)
                    AV_step(jt)

                # evict + normalize: row 64 holds the softmax denominator
                hb = (h % 2) * HD
                for i2 in range(2):
                    rr = bc_pool.tile([1, 512], f32, tag="rr",
                                      name=f"rr_{half}_{h}_{i2}")
                    nc.vector.reciprocal(rr, pav[i2][HD:HD + 1])
                    rb = bc_pool.tile([HD, 512], f32, tag="rb",
                                      name=f"rb_{half}_{h}_{i2}")
                    nc.gpsimd.partition_broadcast(rb, rr)
                    nc.vector.scalar_tensor_tensor(
                        out_normT[hb:hb + HD, h // 2,
                                  colb + i2 * 512:colb + (i2 + 1) * 512],
                        pav[i2][:HD], 0.0, rb, op0=ALU.add, op1=ALU.mult)

            enter_scope("attnP")
            for h in range(NH):
                attn_head(1, h)

            # ---------------- pair exchange (fires under attnM) ----------
            enter_scope("exch")
            for pi in range(FL // P):
                nc.sync.dma_start(cc_in[pi * P:(pi + 1) * P, :],
                                  out_normT[:, pi, TH:S])
            nc.gpsimd.collective_compute(
                "AllReduce", ALU.add, replica_groups=PAIRS,
                ins=[cc_in.opt()], outs=[cc_sum.opt()])

            enter_scope("attnM")
            attn_head(0, 0)

            # peer = (mine + peer) - mine; placed here so the DVE waits on
            # the collective only after head 0 of the own half is in flight.
            enter_scope("exch2")
            peer_T = ws.tile([P, FL // P, TH], bf16, tag="wA")
            ta_all = xs.tile([P, FL // P, TH], bf16, tag="xqk",
                             name="cc_stage")
            nc.sync.dma_start(
                ta_all, cc_sum[:, :].rearrange("(a p) t -> p a t", p=P))
            for pi in range(FL // P):
                nc.vector.scalar_tensor_tensor(
                    peer_T[:, pi, :], ta_all[:, pi, :], 0.0,
                    out_normT[:, pi, TH:S],
                    op0=ALU.add, op1=ALU.subtract)

            enter_scope("attnM2")
            for h in range(1, NH):
                attn_head(0, h)

            # ---------------- output projection + layernorm ----------------
            enter_scope("final")
            for it in range(TH // P):
                psf = [avp.tile([P, 512], f32, tag="ps512",
                                name=f"psf_{it}_{f}") for f in range(2)]
                for fcc in range(2):
                    for cc in range(DC):
                        if cc < 4:
                            stat = out_normT[:, cc, it * P:(it + 1) * P]
                        else:
                            stat = peer_T[:, cc - 4, it * P:(it + 1) * P]
                        nc.tensor.matmul(
                            psf[fcc], stat,
                            woT_sb[:, cc, fcc * 512:(fcc + 1) * 512],
                            start=(cc == 0), stop=(cc == DC - 1))
                xln = ln_pool.tile([P, DIM], f32, tag="xln", name=f"xln_{it}")
                for fcc in range(2):
                    nc.vector.scalar_tensor_tensor(
                        xln[:, fcc * 512:(fcc + 1) * 512], psf[fcc], 0.0,
                        bo_bc[:, fcc * 512:(fcc + 1) * 512],
                        op0=ALU.add, op1=ALU.add)
                stats = ln_pool.tile([P, 2, 6], f32, tag="st", name=f"st_{it}")
                for hf in range(2):
                    nc.vector.bn_stats(stats[:, hf, :],
                                       xln[:, hf * 512:(hf + 1) * 512])
                mv = ln_pool.tile([P, 2], f32, tag="mv", name=f"mv_{it}")
                nc.vector.bn_aggr(mv, stats)
                rstd = ln_pool.tile([P, 1], f32, tag="rstd", name=f"rstd_{it}")
                nc.scalar.activation(rstd, mv[:, 1:2], SQRTF, bias=eps_sb)
                nc.vector.reciprocal(rstd, rstd)
                nc.vector.scalar_tensor_tensor(
                    xln, xln, mv[:, 0:1], gamma_bc,
                    op0=ALU.subtract, op1=ALU.mult)
                nc.vector.scalar_tensor_tensor(
                    xln, xln, rstd, beta_bc, op0=ALU.mult, op1=ALU.add)
                nc.sync.dma_start(y_d[it * P:(it + 1) * P, :], xln)
            while scope_stack:
                n0, sid = scope_stack.pop()
                nc.leave_named_scope(n0, sid, False)

    nc.compile()
    return nc


def kernel(q, k, v, Wq, bq, Wk, bk, Wv, bv, Wo, bo, gamma, beta):
    from concourse.bass_utils import run_bass_kernel_spmd

    if "nc" not in _cache:
        _cache["nc"] = _build()
    nc = _cache["nc"]

    q = np.asarray(q, np.float32)
    k = np.asarray(k, np.float32)
    v = np.asarray(v, np.float32)
    Wq = np.asarray(Wq, np.float32); Wk = np.asarray(Wk, np.float32)
    Wv = np.asarray(Wv, np.float32); Wo = np.asarray(Wo, np.float32)
    bf = ml_dtypes.bfloat16
    WoT = Wo.T  # [c, f]
    in_maps = []
    for c in range(NCORES):
        b, g = c // 2, c % 2
        fsl = slice(g * FL, (g + 1) * FL)
        # my token half first (so "my half" is always columns 0:TH)
        xq = q[b].T if g == 0 else np.concatenate(
            [q[b].T[:, TH:], q[b].T[:, :TH]], axis=1)
        # Wo rows rotated: [my features, peer features]
        woT = np.concatenate([WoT[g * FL:(g + 1) * FL],
                              WoT[(1 - g) * FL:(2 - g) * FL]], axis=0)
        in_maps.append({
            "xqT": np.ascontiguousarray(xq).astype(bf),
            "xkT": np.ascontiguousarray(k[b].T).astype(bf),
            "xvT": np.ascontiguousarray(v[b].T).astype(bf),
            "wqT": np.ascontiguousarray(Wq[fsl, :].T).astype(bf),
            "wkT": np.ascontiguousarray(Wk[fsl, :].T).astype(bf),
            "wvT": np.ascontiguousarray(Wv[fsl, :].T).astype(bf),
            "woT": np.ascontiguousarray(woT).astype(bf),
            "bq": np.asarray(bq, np.float32)[fsl],
            "bk": np.asarray(bk, np.float32)[fsl],
            "bv": np.asarray(bv, np.float32)[fsl],
            "bo": np.asarray(bo, np.float32),
            "gamma": np.asarray(gamma, np.float32),
            "beta": np.asarray(beta, np.float32),
        })
    res = run_bass_kernel_spmd(nc, in_maps, list(range(NCORES)),
                               trace=_cache.get("trace", False))
    _cache["last_res"] = res
    y = np.empty((B, S, DIM), np.float32)
    for c in range(NCORES):
        b, g = c // 2, c % 2
        y[b, g * TH:(g + 1) * TH, :] = res.results[c]["y"]
    return y


# revision 15
# speedup vs baseline: 2.0611x; 1.0349x over previous
"""Fused multi-head attention + LayerNorm kernel for 8 Trainium2 NeuronCores.

Problem (hardcoded): B=4, S=2048, DIM=1024, H=16, HD=64; out = LayerNorm(
softmax(q W_q^T (k W_k^T)^T / sqrt(HD)) (v W_v^T) W_o^T + b_o) per reference.

Sharding: core c -> batch b = c//2, head-group g = c%2 (8 heads / 512 features).
The two cores of a pair exchange normalized attention outputs (AllGather over
pairs) so each finalizes half of the tokens.

v2 design notes (vs the earlier baseline):
  - Everything bf16 except PSUM accumulation (f32r v-path removed; f32r moving
    ran at half rate cold and the extra bytes bought nothing).
  - khT stays resident in SBUF (no DRAM round trip between projection and
    attention).
  - Attention inner loop is software-pipelined: scores for jt+2 are issued
    before AV of jt so the PE never sits behind the ScalarE exp.
  - Denominator: ones-column trick -> PSUM row 64; evict does
    reciprocal (DVE) -> partition_broadcast (GpSimd) -> fused multiply on
    eviction (DVE) straight into out_normT.
  - Peer token-half (cols TH:S) is attended FIRST for all heads, the pair
    AllGather fires, and the own-half attention hides the collective +
    peer reconstruction.

Host tricks: each core's q tokens are permuted so "my half" is always columns
0:1024; Wo rows are rotated per core so the [mine, peer] chunk order matches.
"""
import sys

sys.path.insert(0, "/opt/trn_rl_repo")

import numpy as np
import ml_dtypes

B, S, DIM, H, HD = 4, 2048, 1024, 16, 64
NCORES = 8
NH = 8             # heads per core
FL = NH * HD       # 512 local features
EPS = 1e-5
SCALE = HD ** -0.5
P = 128
JT = S // P        # 16
IC = S // 512      # 4
TH = S // 2        # 1024 tokens finalized per core
DC = DIM // P      # 8 contraction chunks
HW = HD + 1        # 65: head block width in vh_aug
VW = NH * HW       # 520 (stationary windows spill into next block)

_cache = {}


def _build():
    import concourse.bass as bass
    import concourse.bacc as bacc
    import concourse.tile as tile
    from concourse import mybir
    f32 = mybir.dt.float32
    f32r = mybir.dt.float32r
    bf16 = mybir.dt.bfloat16
    EXPF = mybir.ActivationFunctionType.Exp
    IDENT = mybir.ActivationFunctionType.Identity
    SQRTF = mybir.ActivationFunctionType.Sqrt
    ALU = mybir.AluOpType

    nc = bacc.Bacc("TRN2", target_bir_lowering=False, debug=False,
                   num_devices=NCORES)

    xqT_d = nc.dram_tensor("xqT", [DIM, S], bf16, kind="ExternalInput")
    xkT_d = nc.dram_tensor("xkT", [DIM, S], bf16, kind="ExternalInput")
    xvT_d = nc.dram_tensor("xvT", [DIM, S], bf16, kind="ExternalInput")
    wqT_d = nc.dram_tensor("wqT", [DIM, FL], bf16, kind="ExternalInput")
    wkT_d = nc.dram_tensor("wkT", [DIM, FL], bf16, kind="ExternalInput")
    wvT_d = nc.dram_tensor("wvT", [DIM, FL], bf16, kind="ExternalInput")
    woT_d = nc.dram_tensor("woT", [DIM, DIM], bf16, kind="ExternalInput")
    bq_d = nc.dram_tensor("bq", [FL], f32, kind="ExternalInput")
    bk_d = nc.dram_tensor("bk", [FL], f32, kind="ExternalInput")
    bv_d = nc.dram_tensor("bv", [FL], f32, kind="ExternalInput")
    bo_d = nc.dram_tensor("bo", [DIM], f32, kind="ExternalInput")
    gamma_d = nc.dram_tensor("gamma", [DIM], f32, kind="ExternalInput")
    beta_d = nc.dram_tensor("beta", [DIM], f32, kind="ExternalInput")
    y_d = nc.dram_tensor("y", [TH, DIM], f32, kind="ExternalOutput")
    dbg_on_d = nc.dram_tensor("dbg_on", [P, FL // P, S], f32,
                              kind="ExternalOutput")
    dbg_pt_d = nc.dram_tensor("dbg_pt", [P, FL // P, TH], f32,
                              kind="ExternalOutput")
    dbg_woA_d = nc.dram_tensor("dbg_woA", [P, 4, DIM], f32,
                               kind="ExternalOutput")
    dbg_woB_d = nc.dram_tensor("dbg_woB", [P, 4, DIM], f32,
                               kind="ExternalOutput")

    PAIRS = [[0, 1], [2, 3], [4, 5], [6, 7]]

    def bcast_ap(ap, parts):
        return bass.AP(tensor=ap.tensor, offset=ap.offset,
                       ap=[[0, parts]] + list(ap.ap))

    with tile.TileContext(nc) as tc:
        import contextlib
        with contextlib.ExitStack() as ctx:
            persist = ctx.enter_context(tc.tile_pool(name="persist", bufs=1))
            ws = ctx.enter_context(tc.tile_pool(name="ws", bufs=1))
            xs = ctx.enter_context(tc.tile_pool(name="xs", bufs=2))
            et_pool = ctx.enter_context(tc.tile_pool(name="et", bufs=3))
            bc_pool = ctx.enter_context(tc.tile_pool(name="bc", bufs=2))
            ln_pool = ctx.enter_context(tc.tile_pool(name="ln", bufs=2))
            dram = ctx.enter_context(
                tc.tile_pool(name="dram", bufs=1, space="DRAM"))
            scp = ctx.enter_context(
                tc.tile_pool(name="scp", bufs=2, space="PSUM"))
            avp = ctx.enter_context(
                tc.tile_pool(name="avp", bufs=4, space="PSUM"))

            # ---------------- persistent state ----------------
            qhT = persist.tile([HD, NH, S], bf16)
            khT = persist.tile([HD, NH, S], bf16)
            vh_aug = persist.tile([P, JT * VW + (P - HW)], bf16)
            out_normT = persist.tile([P, FL // P, S], bf16)

            # ones columns (rest of vh_aug holds data or harmless garbage;
            # garbage feeds only psum rows 65:127 which are never read)
            ones_f32 = persist.tile([P, P - HW], f32)
            nc.vector.memset(ones_f32, 1.0)
            vh_view = vh_aug[:, :JT * VW].rearrange("p (j w) -> p j w", w=VW)
            for h in range(NH):
                nc.scalar.copy(vh_view[:, :, h * HW + HD], ones_f32[:, :JT])
            # tail pad after the last head's window must be finite
            nc.scalar.copy(vh_aug[:, JT * VW:], ones_f32)

            bq_sb = persist.tile([P, FL // P], f32)
            bk_sb = persist.tile([P, FL // P], f32)
            nc.sync.dma_start(bq_sb, bq_d.rearrange("(a p) -> p a", p=P))
            nc.sync.dma_start(bk_sb, bk_d.rearrange("(a p) -> p a", p=P))
            bv_bc = persist.tile([P, FL], f32)
            nc.gpsimd.dma_start(bv_bc, bcast_ap(bv_d[:], P))
            bo_bc = persist.tile([P, DIM], f32)
            nc.gpsimd.dma_start(bo_bc, bcast_ap(bo_d[:], P))
            gamma_bc = persist.tile([P, DIM], f32)
            nc.gpsimd.dma_start(gamma_bc, bcast_ap(gamma_d[:], P))
            beta_bc = persist.tile([P, DIM], f32)
            nc.gpsimd.dma_start(beta_bc, bcast_ap(beta_d[:], P))
            eps_sb = persist.tile([P, 1], f32)
            nc.vector.memset(eps_sb, EPS)

            wq_sb = ws.tile([P, DC, FL], bf16, tag="wA")
            wk_sb = ws.tile([P, DC, FL], bf16, tag="wB")
            wv_sb = ws.tile([P, DC, FL], bf16, tag="wC")
            nc.sync.dma_start(wq_sb, wqT_d.rearrange("(a p) f -> p a f", p=P))
            nc.sync.dma_start(wk_sb, wkT_d.rearrange("(a p) f -> p a f", p=P))
            nc.sync.dma_start(wv_sb, wvT_d.rearrange("(a p) f -> p a f", p=P))

            scope_stack = []

            def enter_scope(nm):
                while scope_stack:
                    n0, sid = scope_stack.pop()
                    nc.leave_named_scope(n0, sid, False)
                scope_stack.append((nm, nc.enter_named_scope(nm, False)[0]))

            # ---------------- q/k projections ----------------
            enter_scope("proj")
            for name, x_d, w_sb, b_sb, dstT in (
                ("q", xqT_d, wq_sb, bq_sb, qhT),
                ("k", xkT_d, wk_sb, bk_sb, khT),
            ):
                for tci in range(IC):
                    xt = xs.tile([P, DC, 512], bf16, tag="xqk",
                                 name=f"x{name}_{tci}")
                    for dc in range(DC):
                        nc.sync.dma_start(
                            xt[:, dc, :],
                            x_d[dc * P:(dc + 1) * P,
                                tci * 512:(tci + 1) * 512])
                    for fc in range(FL // P):
                        ps = avp.tile([P, 512], f32, tag="ps512",
                                      name=f"ps_{name}_{tci}_{fc}")
                        for dc in range(DC):
                            nc.tensor.matmul(
                                ps, w_sb[:, dc, fc * P:(fc + 1) * P],
                                xt[:, dc, :],
                                start=(dc == 0), stop=(dc == DC - 1))
                        for hf in range(2):
                            h = 2 * fc + hf
                            nc.scalar.activation(
                                dstT[:, h, tci * 512:(tci + 1) * 512],
                                ps[hf * HD:(hf + 1) * HD], IDENT,
                                bias=b_sb[hf * HD:(hf + 1) * HD, fc:fc + 1])

            # woT reuses the q/k weight slots now that they are free
            woA = ws.tile([P, 4, DIM], bf16, tag="wA", name="woA")
            woB = ws.tile([P, 4, DIM], bf16, tag="wB", name="woB")
            for a in range(4):
                nc.sync.dma_start(woA[:, a, :],
                                  woT_d[a * P:(a + 1) * P, :])
                nc.sync.dma_start(woB[:, a, :],
                                  woT_d[FL + a * P:FL + (a + 1) * P, :])

            # ---------------- v projection ----------------
            vh3 = vh_aug[:, :JT * VW].rearrange(
                "p (j h w) -> p j h w", h=NH, w=HW)
            bv3 = bv_bc.rearrange("p (h w) -> p h w", w=HD)
            for tci in range(IC):
                xvt = xs.tile([P, DC, 512], bf16, tag="xqk",
                              name=f"xv_{tci}")
                for dc in range(DC):
                    nc.sync.dma_start(
                        xvt[:, dc, :],
                        xvT_d[dc * P:(dc + 1) * P,
                              tci * 512:(tci + 1) * 512])
                for tj in range(4):
                    jt = tci * 4 + tj
                    ps = avp.tile([P, FL], f32, tag="ps512",
                                  name=f"ps_v_{jt}")
                    for dc in range(DC):
                        nc.tensor.matmul(
                            ps, xvt[:, dc, tj * P:(tj + 1) * P],
                            wv_sb[:, dc, :],
                            start=(dc == 0), stop=(dc == DC - 1))
                    nc.vector.scalar_tensor_tensor(
                        vh3[:, jt, :, :HD],
                        ps.rearrange("p (h w) -> p h w", w=HD), 0.0,
                        bv3, op0=ALU.add, op1=ALU.add)

            # ---------------- attention ----------------
            # half=1 (peer token half, cols TH:S) first so the pair exchange
            # overlaps the half=0 attention.
            cc_in = dram.tile([FL, TH], bf16)
            cc_sum = dram.tile([FL, TH], bf16)

            def attn_head(half, h):
                colb = half * TH
                pav = [avp.tile([P, 512], f32, tag="ps512",
                                name=f"pav_{half}_{h}_{i}") for i in range(2)]
                et_t = {}

                def S_step(jt):
                    psc = scp.tile([P, 1024], f32, tag="sc",
                                   name=f"sc_{half}_{h}_{jt}")
                    for i2 in range(2):
                        nc.tensor.matmul(
                            psc[:, i2 * 512:(i2 + 1) * 512],
                            khT[:, h, jt * P:(jt + 1) * P],
                            qhT[:, h, colb + i2 * 512:colb + (i2 + 1) * 512],
                            start=True, stop=True)
                    et = et_pool.tile([P, 1024], bf16, tag="et",
                                      name=f"et_{half}_{h}_{jt}")
                    nc.scalar.activation(et, psc, EXPF, scale=SCALE)
                    et_t[jt] = et

                def AV_step(jt):
                    et = et_t.pop(jt)
                    for i2 in range(2):
                        nc.tensor.matmul(
                            pav[i2],
                            vh_aug[:, jt * VW + h * HW:jt * VW + h * HW + P],
                            et[:, i2 * 512:(i2 + 1) * 512],
                            start=(jt == 0), stop=(jt == JT - 1))

                S_step(0)
                S_step(1)
                for jt in range(JT):
                    if jt + 2 < JT:
                        S_step(jt + 2)
                    AV_step(jt)

                # evict + normalize: row 64 holds the softmax denominator
                hb = (h % 2) * HD
                for i2 in range(2):
                    rr = bc_pool.tile([1, 2, 512], f32, tag="rr",
                                      name=f"rr_{half}_{h}_{i2}")
                    # custom-DVE recip can't read PSUM: stage row to SBUF
                    nc.vector.tensor_copy(rr[:, 1, :], pav[i2][HD:HD + 1])
                    nc.vector.reciprocal_approx_fast(rr[:, 0, :], rr[:, 1, :])
                    rb = bc_pool.tile([HD, 512], f32, tag="rb",
                                      name=f"rb_{half}_{h}_{i2}")
                    nc.gpsimd.partition_broadcast(rb, rr[:, 0, :])
                    nc.vector.scalar_tensor_tensor(
                        out_normT[hb:hb + HD, h // 2,
                                  colb + i2 * 512:colb + (i2 + 1) * 512],
                        pav[i2][:HD], 0.0, rb, op0=ALU.add, op1=ALU.mult)

            enter_scope("attnP")
            for h in range(NH):
                attn_head(1, h)

            # ---------------- pair exchange (fires under attnM) ----------
            enter_scope("exch")
            for pi in range(FL // P):
                nc.sync.dma_start(cc_in[pi * P:(pi + 1) * P, :],
                                  out_normT[:, pi, TH:S])
            nc.gpsimd.collective_compute(
                "AllReduce", ALU.add, replica_groups=PAIRS,
                ins=[cc_in.opt()], outs=[cc_sum.opt()])

            enter_scope("attnM2")
            peer_T = None
            for h in range(NH):
                attn_head(0, h)
                if h == 4:
                    # peer = (mine + peer) - mine; emitted this late so the
                    # collective is long done and the DVE never head-of-line
                    # blocks the eviction chain (a stall here re-throttles
                    # the PE clock for the rest of the kernel).
                    peer_T = ws.tile([P, FL // P, TH], bf16, tag="wC")
                    ta_all = xs.tile([P, FL // P, TH], bf16, tag="xqk",
                                     name="cc_stage")
                    nc.sync.dma_start(
                        ta_all,
                        cc_sum[:, :].rearrange("(a p) t -> p a t", p=P))
                    for pi in range(FL // P):
                        nc.vector.scalar_tensor_tensor(
                            peer_T[:, pi, :], ta_all[:, pi, :], 0.0,
                            out_normT[:, pi, TH:S],
                            op0=ALU.add, op1=ALU.subtract)

            # ---------------- output projection + layernorm ----------------
            enter_scope("final")
            for it in range(TH // P):
                psf = [avp.tile([P, 512], f32, tag="ps512",
                                name=f"psf_{it}_{f}") for f in range(2)]
                for fcc in range(2):
                    for cc in range(DC):
                        if cc < 4:
                            stat = out_normT[:, cc, it * P:(it + 1) * P]
                            mov = woA[:, cc, fcc * 512:(fcc + 1) * 512]
                        else:
                            stat = peer_T[:, cc - 4, it * P:(it + 1) * P]
                            mov = woB[:, cc - 4, fcc * 512:(fcc + 1) * 512]
                        nc.tensor.matmul(
                            psf[fcc], stat, mov,
                            start=(cc == 0), stop=(cc == DC - 1))
                xln = ln_pool.tile([P, DIM], f32, tag="xln", name=f"xln_{it}")
                for fcc in range(2):
                    nc.vector.scalar_tensor_tensor(
                        xln[:, fcc * 512:(fcc + 1) * 512], psf[fcc], 0.0,
                        bo_bc[:, fcc * 512:(fcc + 1) * 512],
                        op0=ALU.add, op1=ALU.add)
                stats = ln_pool.tile([P, 2, 6], f32, tag="st", name=f"st_{it}")
                for hf in range(2):
                    nc.vector.bn_stats(stats[:, hf, :],
                                       xln[:, hf * 512:(hf + 1) * 512])
                mv = ln_pool.tile([P, 2], f32, tag="mv", name=f"mv_{it}")
                nc.vector.bn_aggr(mv, stats)
                rstd = ln_pool.tile([P, 1], f32, tag="rstd", name=f"rstd_{it}")
                nc.scalar.activation(rstd, mv[:, 1:2], SQRTF, bias=eps_sb)
                nc.vector.reciprocal(rstd, rstd)
                nc.vector.scalar_tensor_tensor(
                    xln, xln, mv[:, 0:1], gamma_bc,
                    op0=ALU.subtract, op1=ALU.mult)
                nc.vector.scalar_tensor_tensor(
                    xln, xln, rstd, beta_bc, op0=ALU.mult, op1=ALU.add)
                nc.sync.dma_start(y_d[it * P:(it + 1) * P, :], xln)
            while scope_stack:
                n0, sid = scope_stack.pop()
                nc.leave_named_scope(n0, sid, False)

    nc.compile()
    return nc


def kernel(q, k, v, Wq, bq, Wk, bk, Wv, bv, Wo, bo, gamma, beta):
    from concourse.bass_utils import run_bass_kernel_spmd

    if "nc" not in _cache:
        _cache["nc"] = _build()
    nc = _cache["nc"]

    q = np.asarray(q, np.float32)
    k = np.asarray(k, np.float32)
    v = np.asarray(v, np.float32)
    Wq = np.asarray(Wq, np.float32); Wk = np.asarray(Wk, np.float32)
    Wv = np.asarray(Wv, np.float32); Wo = np.asarray(Wo, np.float32)
    bf = ml_dtypes.bfloat16
    WoT = Wo.T  # [c, f]
    in_maps = []
    for c in range(NCORES):
        b, g = c // 2, c % 2
        fsl = slice(g * FL, (g + 1) * FL)
        # my token half first (so "my half" is always columns 0:TH)
        xq = q[b].T if g == 0 else np.concatenate(
            [q[b].T[:, TH:], q[b].T[:, :TH]], axis=1)
        # Wo rows rotated: [my features, peer features]
        woT = np.concatenate([WoT[g * FL:(g + 1) * FL],
                              WoT[(1 - g) * FL:(2 - g) * FL]], axis=0)
        in_maps.append({
            "xqT": np.ascontiguousarray(xq).astype(bf),
            "xkT": np.ascontiguousarray(k[b].T).astype(bf),
            "xvT": np.ascontiguousarray(v[b].T).astype(bf),
            "wqT": np.ascontiguousarray(Wq[fsl, :].T).astype(bf),
            "wkT": np.ascontiguousarray(Wk[fsl, :].T).astype(bf),
            "wvT": np.ascontiguousarray(Wv[fsl, :].T).astype(bf),
            "woT": np.ascontiguousarray(woT).astype(bf),
            "bq": np.asarray(bq, np.float32)[fsl],
            "bk": np.asarray(bk, np.float32)[fsl],
            "bv": np.asarray(bv, np.float32)[fsl],
            "bo": np.asarray(bo, np.float32),
            "gamma": np.asarray(gamma, np.float32),
            "beta": np.asarray(beta, np.float32),
        })
    res = run_bass_kernel_spmd(nc, in_maps, list(range(NCORES)),
                               trace=_cache.get("trace", False))
    _cache["last_res"] = res
    y = np.empty((B, S, DIM), np.float32)
    for c in range(NCORES):
        b, g = c // 2, c % 2
        y[b, g * TH:(g + 1) * TH, :] = res.results[c]["y"]
    return y


# revision 16
# speedup vs baseline: 2.0648x; 1.0018x over previous
"""Fused multi-head attention + LayerNorm kernel for 8 Trainium2 NeuronCores.

Problem (hardcoded): B=4, S=2048, DIM=1024, H=16, HD=64; out = LayerNorm(
softmax(q W_q^T (k W_k^T)^T / sqrt(HD)) (v W_v^T) W_o^T + b_o) per reference.

Sharding: core c -> batch b = c//2, head-group g = c%2 (8 heads / 512 features).
The two cores of a pair exchange normalized attention outputs (AllGather over
pairs) so each finalizes half of the tokens.

v2 design notes (vs the earlier baseline):
  - Everything bf16 except PSUM accumulation (f32r v-path removed; f32r moving
    ran at half rate cold and the extra bytes bought nothing).
  - khT stays resident in SBUF (no DRAM round trip between projection and
    attention).
  - Attention inner loop is software-pipelined: scores for jt+2 are issued
    before AV of jt so the PE never sits behind the ScalarE exp.
  - Denominator: ones-column trick -> PSUM row 64; evict does
    reciprocal (DVE) -> partition_broadcast (GpSimd) -> fused multiply on
    eviction (DVE) straight into out_normT.
  - Peer token-half (cols TH:S) is attended FIRST for all heads, the pair
    AllGather fires, and the own-half attention hides the collective +
    peer reconstruction.

Host tricks: each core's q tokens are permuted so "my half" is always columns
0:1024; Wo rows are rotated per core so the [mine, peer] chunk order matches.
"""
import sys

sys.path.insert(0, "/opt/trn_rl_repo")

import numpy as np
import ml_dtypes

B, S, DIM, H, HD = 4, 2048, 1024, 16, 64
NCORES = 8
NH = 8             # heads per core
FL = NH * HD       # 512 local features
EPS = 1e-5
SCALE = HD ** -0.5
P = 128
JT = S // P        # 16
IC = S // 512      # 4
TH = S // 2        # 1024 tokens finalized per core
DC = DIM // P      # 8 contraction chunks
HW = HD + 1        # 65: head block width in vh_aug
VW = NH * HW       # 520 (stationary windows spill into next block)

_cache = {}


def _build():
    import concourse.bass as bass
    import concourse.bacc as bacc
    import concourse.tile as tile
    from concourse import mybir
    f32 = mybir.dt.float32
    f32r = mybir.dt.float32r
    bf16 = mybir.dt.bfloat16
    EXPF = mybir.ActivationFunctionType.Exp
    IDENT = mybir.ActivationFunctionType.Identity
    SQRTF = mybir.ActivationFunctionType.Sqrt
    ALU = mybir.AluOpType

    nc = bacc.Bacc("TRN2", target_bir_lowering=False, debug=False,
                   num_devices=NCORES)

    xqT_d = nc.dram_tensor("xqT", [DIM, S], bf16, kind="ExternalInput")
    xkT_d = nc.dram_tensor("xkT", [DIM, S], bf16, kind="ExternalInput")
    xvT_d = nc.dram_tensor("xvT", [DIM, S], bf16, kind="ExternalInput")
    wqT_d = nc.dram_tensor("wqT", [DIM, FL], bf16, kind="ExternalInput")
    wkT_d = nc.dram_tensor("wkT", [DIM, FL], bf16, kind="ExternalInput")
    wvT_d = nc.dram_tensor("wvT", [DIM, FL], bf16, kind="ExternalInput")
    woT_d = nc.dram_tensor("woT", [DIM, DIM], bf16, kind="ExternalInput")
    bq_d = nc.dram_tensor("bq", [FL], f32, kind="ExternalInput")
    bk_d = nc.dram_tensor("bk", [FL], f32, kind="ExternalInput")
    bv_d = nc.dram_tensor("bv", [FL], f32, kind="ExternalInput")
    bo_d = nc.dram_tensor("bo", [DIM], f32, kind="ExternalInput")
    gamma_d = nc.dram_tensor("gamma", [DIM], f32, kind="ExternalInput")
    beta_d = nc.dram_tensor("beta", [DIM], f32, kind="ExternalInput")
    y_d = nc.dram_tensor("y", [TH, DIM], f32, kind="ExternalOutput")

    PAIRS = [[0, 1], [2, 3], [4, 5], [6, 7]]

    def bcast_ap(ap, parts):
        return bass.AP(tensor=ap.tensor, offset=ap.offset,
                       ap=[[0, parts]] + list(ap.ap))

    with tile.TileContext(nc) as tc:
        import contextlib
        with contextlib.ExitStack() as ctx:
            persist = ctx.enter_context(tc.tile_pool(name="persist", bufs=1))
            ws = ctx.enter_context(tc.tile_pool(name="ws", bufs=1))
            xs = ctx.enter_context(tc.tile_pool(name="xs", bufs=2))
            et_pool = ctx.enter_context(tc.tile_pool(name="et", bufs=3))
            bc_pool = ctx.enter_context(tc.tile_pool(name="bc", bufs=2))
            ln_pool = ctx.enter_context(tc.tile_pool(name="ln", bufs=2))
            dram = ctx.enter_context(
                tc.tile_pool(name="dram", bufs=1, space="DRAM"))
            scp = ctx.enter_context(
                tc.tile_pool(name="scp", bufs=2, space="PSUM"))
            avp = ctx.enter_context(
                tc.tile_pool(name="avp", bufs=4, space="PSUM"))

            # ---------------- persistent state ----------------
            qhT = persist.tile([HD, NH, S], bf16)
            khT = persist.tile([HD, NH, S], bf16)
            vh_aug = persist.tile([P, JT * VW + (P - HW)], bf16)
            out_normT = persist.tile([P, FL // P, S], bf16)

            wq_sb = ws.tile([P, DC, FL], bf16, tag="wA")
            wk_sb = ws.tile([P, DC, FL], bf16, tag="wB")
            wv_sb = ws.tile([P, DC, FL], bf16, tag="wC")
            nc.sync.dma_start(wq_sb, wqT_d.rearrange("(a p) f -> p a f", p=P))
            nc.sync.dma_start(wk_sb, wkT_d.rearrange("(a p) f -> p a f", p=P))
            nc.sync.dma_start(wv_sb, wvT_d.rearrange("(a p) f -> p a f", p=P))

            # ones columns (rest of vh_aug holds data or harmless garbage;
            # garbage feeds only psum rows 65:127 which are never read)
            ones_f32 = persist.tile([P, P - HW], f32)
            nc.vector.memset(ones_f32, 1.0)
            vh_view = vh_aug[:, :JT * VW].rearrange("p (j w) -> p j w", w=VW)
            for h in range(NH):
                nc.scalar.copy(vh_view[:, :, h * HW + HD], ones_f32[:, :JT])
            # tail pad after the last head's window must be finite
            nc.scalar.copy(vh_aug[:, JT * VW:], ones_f32)

            bq_sb = persist.tile([P, FL // P], f32)
            bk_sb = persist.tile([P, FL // P], f32)
            nc.sync.dma_start(bq_sb, bq_d.rearrange("(a p) -> p a", p=P))
            nc.sync.dma_start(bk_sb, bk_d.rearrange("(a p) -> p a", p=P))
            bv_bc = persist.tile([P, FL], f32)
            nc.gpsimd.dma_start(bv_bc, bcast_ap(bv_d[:], P))
            bo_bc = persist.tile([P, DIM], f32)
            gamma_bc = persist.tile([P, DIM], f32)
            beta_bc = persist.tile([P, DIM], f32)
            eps_sb = persist.tile([P, 1], f32)
            nc.vector.memset(eps_sb, EPS)

            scope_stack = []

            def enter_scope(nm):
                while scope_stack:
                    n0, sid = scope_stack.pop()
                    nc.leave_named_scope(n0, sid, False)
                scope_stack.append((nm, nc.enter_named_scope(nm, False)[0]))

            # ---------------- q/k projections ----------------
            enter_scope("proj")
            for name, x_d, w_sb, b_sb, dstT in (
                ("q", xqT_d, wq_sb, bq_sb, qhT),
                ("k", xkT_d, wk_sb, bk_sb, khT),
            ):
                for tci in range(IC):
                    xt = xs.tile([P, DC, 512], bf16, tag="xqk",
                                 name=f"x{name}_{tci}")
                    for dc in range(DC):
                        nc.sync.dma_start(
                            xt[:, dc, :],
                            x_d[dc * P:(dc + 1) * P,
                                tci * 512:(tci + 1) * 512])
                    for fc in range(FL // P):
                        ps = avp.tile([P, 512], f32, tag="ps512",
                                      name=f"ps_{name}_{tci}_{fc}")
                        for dc in range(DC):
                            nc.tensor.matmul(
                                ps, w_sb[:, dc, fc * P:(fc + 1) * P],
                                xt[:, dc, :],
                                start=(dc == 0), stop=(dc == DC - 1))
                        for hf in range(2):
                            h = 2 * fc + hf
                            nc.scalar.activation(
                                dstT[:, h, tci * 512:(tci + 1) * 512],
                                ps[hf * HD:(hf + 1) * HD], IDENT,
                                bias=b_sb[hf * HD:(hf + 1) * HD, fc:fc + 1])

            # woT reuses the q/k weight slots now that they are free
            woA = ws.tile([P, 4, DIM], bf16, tag="wA", name="woA")
            woB = ws.tile([P, 4, DIM], bf16, tag="wB", name="woB")
            for a in range(4):
                nc.sync.dma_start(woA[:, a, :],
                                  woT_d[a * P:(a + 1) * P, :])
                nc.sync.dma_start(woB[:, a, :],
                                  woT_d[FL + a * P:FL + (a + 1) * P, :])

            # ---------------- v projection ----------------
            vh3 = vh_aug[:, :JT * VW].rearrange(
                "p (j h w) -> p j h w", h=NH, w=HW)
            bv3 = bv_bc.rearrange("p (h w) -> p h w", w=HD)
            for tci in range(IC):
                xvt = xs.tile([P, DC, 512], bf16, tag="xqk",
                              name=f"xv_{tci}")
                for dc in range(DC):
                    nc.sync.dma_start(
                        xvt[:, dc, :],
                        xvT_d[dc * P:(dc + 1) * P,
                              tci * 512:(tci + 1) * 512])
                for tj in range(4):
                    jt = tci * 4 + tj
                    ps = avp.tile([P, FL], f32, tag="ps512",
                                  name=f"ps_v_{jt}")
                    for dc in range(DC):
                        nc.tensor.matmul(
                            ps, xvt[:, dc, tj * P:(tj + 1) * P],
                            wv_sb[:, dc, :],
                            start=(dc == 0), stop=(dc == DC - 1))
                    nc.vector.scalar_tensor_tensor(
                        vh3[:, jt, :, :HD],
                        ps.rearrange("p (h w) -> p h w", w=HD), 0.0,
                        bv3, op0=ALU.add, op1=ALU.add)

            # ---------------- attention ----------------
            # half=1 (peer token half, cols TH:S) first so the pair exchange
            # overlaps the half=0 attention.
            cc_in = dram.tile([FL, TH], bf16)
            cc_sum = dram.tile([FL, TH], bf16)

            def attn_head(half, h):
                colb = half * TH
                pav = [avp.tile([P, 512], f32, tag="ps512",
                                name=f"pav_{half}_{h}_{i}") for i in range(2)]
                et_t = {}

                def S_step(jt):
                    psc = scp.tile([P, 1024], f32, tag="sc",
                                   name=f"sc_{half}_{h}_{jt}")
                    for i2 in range(2):
                        nc.tensor.matmul(
                            psc[:, i2 * 512:(i2 + 1) * 512],
                            khT[:, h, jt * P:(jt + 1) * P],
                            qhT[:, h, colb + i2 * 512:colb + (i2 + 1) * 512],
                            start=True, stop=True)
                    et = et_pool.tile([P, 1024], bf16, tag="et",
                                      name=f"et_{half}_{h}_{jt}")
                    nc.scalar.activation(et, psc, EXPF, scale=SCALE)
                    et_t[jt] = et

                def AV_step(jt):
                    et = et_t.pop(jt)
                    for i2 in range(2):
                        nc.tensor.matmul(
                            pav[i2],
                            vh_aug[:, jt * VW + h * HW:jt * VW + h * HW + P],
                            et[:, i2 * 512:(i2 + 1) * 512],
                            start=(jt == 0), stop=(jt == JT - 1))

                S_step(0)
                S_step(1)
                for jt in range(JT):
                    if jt + 2 < JT:
                        S_step(jt + 2)
                    AV_step(jt)

                # evict + normalize: row 64 holds the softmax denominator
                hb = (h % 2) * HD
                for i2 in range(2):
                    rr = bc_pool.tile([1, 2, 512], f32, tag="rr",
                                      name=f"rr_{half}_{h}_{i2}")
                    # custom-DVE recip can't read PSUM: stage row to SBUF
                    nc.vector.tensor_copy(rr[:, 1, :], pav[i2][HD:HD + 1])
                    nc.vector.reciprocal_approx_fast(rr[:, 0, :], rr[:, 1, :])
                    rb = bc_pool.tile([HD, 512], f32, tag="rb",
                                      name=f"rb_{half}_{h}_{i2}")
                    nc.gpsimd.partition_broadcast(rb, rr[:, 0, :])
                    nc.vector.scalar_tensor_tensor(
                        out_normT[hb:hb + HD, h // 2,
                                  colb + i2 * 512:colb + (i2 + 1) * 512],
                        pav[i2][:HD], 0.0, rb, op0=ALU.add, op1=ALU.mult)

            enter_scope("attnP")
            for h in range(NH):
                attn_head(1, h)

            # ---------------- pair exchange (fires under attnM) ----------
            enter_scope("exch")
            for pi in range(FL // P):
                nc.sync.dma_start(cc_in[pi * P:(pi + 1) * P, :],
                                  out_normT[:, pi, TH:S])
            nc.gpsimd.collective_compute(
                "AllReduce", ALU.add, replica_groups=PAIRS,
                ins=[cc_in.opt()], outs=[cc_sum.opt()])

            nc.gpsimd.dma_start(bo_bc, bcast_ap(bo_d[:], P))
            nc.gpsimd.dma_start(gamma_bc, bcast_ap(gamma_d[:], P))
            nc.gpsimd.dma_start(beta_bc, bcast_ap(beta_d[:], P))

            enter_scope("attnM2")
            peer_T = None
            for h in range(NH):
                attn_head(0, h)
                if h == 4:
                    # peer = (mine + peer) - mine; emitted this late so the
                    # collective is long done and the DVE never head-of-line
                    # blocks the eviction chain (a stall here re-throttles
                    # the PE clock for the rest of the kernel).
                    peer_T = ws.tile([P, FL // P, TH], bf16, tag="wC")
                    ta_all = xs.tile([P, FL // P, TH], bf16, tag="xqk",
                                     name="cc_stage")
                    nc.sync.dma_start(
                        ta_all,
                        cc_sum[:, :].rearrange("(a p) t -> p a t", p=P))
                    for pi in range(FL // P):
                        nc.vector.scalar_tensor_tensor(
                            peer_T[:, pi, :], ta_all[:, pi, :], 0.0,
                            out_normT[:, pi, TH:S],
                            op0=ALU.add, op1=ALU.subtract)

            # ---------------- output projection + layernorm ----------------
            enter_scope("final")
            for it in range(TH // P):
                psf = [avp.tile([P, 512], f32, tag="ps512",
                                name=f"psf_{it}_{f}") for f in range(2)]
                for fcc in range(2):
                    for cc in range(DC):
                        if cc < 4:
                            stat = out_normT[:, cc, it * P:(it + 1) * P]
                            mov = woA[:, cc, fcc * 512:(fcc + 1) * 512]
                        else:
                            stat = peer_T[:, cc - 4, it * P:(it + 1) * P]
                            mov = woB[:, cc - 4, fcc * 512:(fcc + 1) * 512]
                        nc.tensor.matmul(
                            psf[fcc], stat, mov,
                            start=(cc == 0), stop=(cc == DC - 1))
                xln = ln_pool.tile([P, DIM], f32, tag="xln", name=f"xln_{it}")
                for fcc in range(2):
                    nc.vector.scalar_tensor_tensor(
                        xln[:, fcc * 512:(fcc + 1) * 512], psf[fcc], 0.0,
                        bo_bc[:, fcc * 512:(fcc + 1) * 512],
                        op0=ALU.add, op1=ALU.add)
                stats = ln_pool.tile([P, 2, 6], f32, tag="st", name=f"st_{it}")
                for hf in range(2):
                    nc.vector.bn_stats(stats[:, hf, :],
                                       xln[:, hf * 512:(hf + 1) * 512])
                mv = ln_pool.tile([P, 2], f32, tag="mv", name=f"mv_{it}")
                nc.vector.bn_aggr(mv, stats)
                rstd = ln_pool.tile([P, 1], f32, tag="rstd", name=f"rstd_{it}")
                nc.scalar.activation(rstd, mv[:, 1:2], SQRTF, bias=eps_sb)
                nc.vector.reciprocal(rstd, rstd)
                nc.vector.scalar_tensor_tensor(
                    xln, xln, mv[:, 0:1], gamma_bc,
                    op0=ALU.subtract, op1=ALU.mult)
                nc.vector.scalar_tensor_tensor(
                    xln, xln, rstd, beta_bc, op0=ALU.mult, op1=ALU.add)
                nc.sync.dma_start(y_d[it * P:(it + 1) * P, :], xln)
            while scope_stack:
                n0, sid = scope_stack.pop()
                nc.leave_named_scope(n0, sid, False)

    nc.compile()
    return nc


def kernel(q, k, v, Wq, bq, Wk, bk, Wv, bv, Wo, bo, gamma, beta):
    from concourse.bass_utils import run_bass_kernel_spmd

    if "nc" not in _cache:
        _cache["nc"] = _build()
    nc = _cache["nc"]

    q = np.asarray(q, np.float32)
    k = np.asarray(k, np.float32)
    v = np.asarray(v, np.float32)
    Wq = np.asarray(Wq, np.float32); Wk = np.asarray(Wk, np.float32)
    Wv = np.asarray(Wv, np.float32); Wo = np.asarray(Wo, np.float32)
    bf = ml_dtypes.bfloat16
    WoT = Wo.T  # [c, f]
    in_maps = []
    for c in range(NCORES):
        b, g = c // 2, c % 2
        fsl = slice(g * FL, (g + 1) * FL)
        # my token half first (so "my half" is always columns 0:TH)
        xq = q[b].T if g == 0 else np.concatenate(
            [q[b].T[:, TH:], q[b].T[:, :TH]], axis=1)
        # Wo rows rotated: [my features, peer features]
        woT = np.concatenate([WoT[g * FL:(g + 1) * FL],
                              WoT[(1 - g) * FL:(2 - g) * FL]], axis=0)
        in_maps.append({
            "xqT": np.ascontiguousarray(xq).astype(bf),
            "xkT": np.ascontiguousarray(k[b].T).astype(bf),
            "xvT": np.ascontiguousarray(v[b].T).astype(bf),
            "wqT": np.ascontiguousarray(Wq[fsl, :].T).astype(bf),
            "wkT": np.ascontiguousarray(Wk[fsl, :].T).astype(bf),
            "wvT": np.ascontiguousarray(Wv[fsl, :].T).astype(bf),
            "woT": np.ascontiguousarray(woT).astype(bf),
            "bq": np.asarray(bq, np.float32)[fsl],
            "bk": np.asarray(bk, np.float32)[fsl],
            "bv": np.asarray(bv, np.float32)[fsl],
            "bo": np.asarray(bo, np.float32),
            "gamma": np.asarray(gamma, np.float32),
            "beta": np.asarray(beta, np.float32),
        })
    res = run_bass_kernel_spmd(nc, in_maps, list(range(NCORES)),
                               trace=_cache.get("trace", False))
    _cache["last_res"] = res
    y = np.empty((B, S, DIM), np.float32)
    for c in range(NCORES):
        b, g = c // 2, c % 2
        y[b, g * TH:(g + 1) * TH, :] = res.results[c]["y"]
    return y


# revision 18
# speedup vs baseline: 2.4124x; 1.1684x over previous
"""Fused multi-head attention + LayerNorm kernel for 8 Trainium2 NeuronCores.

Problem (hardcoded): B=4, S=2048, DIM=1024, H=16, HD=64; out = LayerNorm(
softmax(q W_q^T (k W_k^T)^T / sqrt(HD)) (v W_v^T) W_o^T + b_o) per reference.

Sharding: core c -> batch b = c//2, head-group g = c%2 (8 heads / 512 features).
Each core attends its 8 heads over all 2048 tokens; the pair exchanges
normalized attention outputs (AllReduce over pairs, sum-minus-mine) so each
core finalizes half of the tokens through the output projection + LayerNorm.

Design notes:
  - All-bf16 datapath (PSUM accumulates f32); khT/qhT stay resident in SBUF.
  - Attention inner loop is software-pipelined: scores for jt+2 issue before
    AV of jt, so the PE never waits on the ScalarE exp; psc double-buffered,
    et triple-buffered; PE runs back-to-back (HAM stays warm).
  - Softmax denominator via a ones-column in the packed V operand (row 64 of
    the AV accumulator); evict = copy row to SBUF -> reciprocal_approx_fast
    (the custom-DVE op cannot read PSUM) -> partition_broadcast -> fused
    multiply on eviction into out_normT.
  - Peer token-half (cols TH:S) is attended first for all heads, the pair
    AllReduce fires, then the own half runs; the peer reconstruction is
    emitted only after own-half head 4 so the DVE never head-of-line blocks
    on the collective (a stall there re-throttles the PE clock down to
    1.2 GHz for the rest of the kernel).
  - Output projection reads [mine (SBUF), peer] with Wo rows rotated
    per-core host-side; LayerNorm via bn_stats/bn_aggr on the DVE.

Host tricks: each core's q tokens are permuted so "my half" is always columns
0:1024; Wo rows are rotated per core so the [mine, peer] chunk order matches.
"""
import sys

sys.path.insert(0, "/opt/trn_rl_repo")

import numpy as np
import ml_dtypes

B, S, DIM, H, HD = 4, 2048, 1024, 16, 64
NCORES = 8
NH = 8             # heads per core
FL = NH * HD       # 512 local features
EPS = 1e-5
SCALE = HD ** -0.5
P = 128
JT = S // P        # 16
IC = S // 512      # 4
TH = S // 2        # 1024 tokens finalized per core
DC = DIM // P      # 8 contraction chunks
HW = HD + 1        # 65: head block width in vh_aug
VW = NH * HW       # 520 (stationary windows spill into next block)

_cache = {}


def _build():
    import concourse.bass as bass
    import concourse.bacc as bacc
    import concourse.tile as tile
    from concourse import mybir
    f32 = mybir.dt.float32
    f32r = mybir.dt.float32r
    bf16 = mybir.dt.bfloat16
    EXPF = mybir.ActivationFunctionType.Exp
    IDENT = mybir.ActivationFunctionType.Identity
    SQRTF = mybir.ActivationFunctionType.Sqrt
    ALU = mybir.AluOpType

    nc = bacc.Bacc("TRN2", target_bir_lowering=False, debug=False,
                   num_devices=NCORES)

    xqT_d = nc.dram_tensor("xqT", [DIM, S], bf16, kind="ExternalInput")
    xkT_d = nc.dram_tensor("xkT", [DIM, S], bf16, kind="ExternalInput")
    xvT_d = nc.dram_tensor("xvT", [DIM, S], bf16, kind="ExternalInput")
    wqT_d = nc.dram_tensor("wqT", [DIM, FL], bf16, kind="ExternalInput")
    wkT_d = nc.dram_tensor("wkT", [DIM, FL], bf16, kind="ExternalInput")
    wvT_d = nc.dram_tensor("wvT", [DIM, FL], bf16, kind="ExternalInput")
    woT_d = nc.dram_tensor("woT", [DIM, DIM], bf16, kind="ExternalInput")
    bq_d = nc.dram_tensor("bq", [FL], f32, kind="ExternalInput")
    bk_d = nc.dram_tensor("bk", [FL], f32, kind="ExternalInput")
    bv_d = nc.dram_tensor("bv", [FL], f32, kind="ExternalInput")
    bo_d = nc.dram_tensor("bo", [DIM], f32, kind="ExternalInput")
    gamma_d = nc.dram_tensor("gamma", [DIM], f32, kind="ExternalInput")
    beta_d = nc.dram_tensor("beta", [DIM], f32, kind="ExternalInput")
    y_d = nc.dram_tensor("y", [TH, DIM], f32, kind="ExternalOutput")

    PAIRS = [[0, 1], [2, 3], [4, 5], [6, 7]]

    def bcast_ap(ap, parts):
        return bass.AP(tensor=ap.tensor, offset=ap.offset,
                       ap=[[0, parts]] + list(ap.ap))

    with tile.TileContext(nc) as tc:
        import contextlib
        with contextlib.ExitStack() as ctx:
            persist = ctx.enter_context(tc.tile_pool(name="persist", bufs=1))
            ws = ctx.enter_context(tc.tile_pool(name="ws", bufs=1))
            xs = ctx.enter_context(tc.tile_pool(name="xs", bufs=2))
            et_pool = ctx.enter_context(tc.tile_pool(name="et", bufs=3))
            bc_pool = ctx.enter_context(tc.tile_pool(name="bc", bufs=2))
            ln_pool = ctx.enter_context(tc.tile_pool(name="ln", bufs=2))
            dram = ctx.enter_context(
                tc.tile_pool(name="dram", bufs=1, space="DRAM"))
            scp = ctx.enter_context(
                tc.tile_pool(name="scp", bufs=2, space="PSUM"))
            avp = ctx.enter_context(
                tc.tile_pool(name="avp", bufs=4, space="PSUM"))

            # ---------------- persistent state ----------------
            qhT = persist.tile([HD, NH, S], bf16)
            khT = persist.tile([HD, NH, S], bf16)
            vh_aug = persist.tile([P, JT * VW + (P - HW)], bf16)
            out_normT = persist.tile([P, FL // P, S], bf16)

            wq_sb = ws.tile([P, DC, FL], bf16, tag="wA")
            wk_sb = ws.tile([P, DC, FL], bf16, tag="wB")
            wv_sb = ws.tile([P, DC, FL], bf16, tag="wC")
            # wq on the sync queue (first proj MM needs it + xq tci0);
            # wk/wv on the gpsimd queue so they load in parallel.
            nc.sync.dma_start(wq_sb, wqT_d.rearrange("(a p) f -> p a f", p=P))
            nc.gpsimd.dma_start(wk_sb,
                                wkT_d.rearrange("(a p) f -> p a f", p=P))
            nc.gpsimd.dma_start(wv_sb,
                                wvT_d.rearrange("(a p) f -> p a f", p=P))

            # ones columns (rest of vh_aug holds data or harmless garbage;
            # garbage feeds only psum rows 65:127 which are never read)
            ones_f32 = persist.tile([P, P - HW], f32)
            nc.vector.memset(ones_f32, 1.0)
            vh_view = vh_aug[:, :JT * VW].rearrange("p (j w) -> p j w", w=VW)
            for h in range(NH):
                nc.scalar.copy(vh_view[:, :, h * HW + HD], ones_f32[:, :JT])
            # tail pad after the last head's window must be finite
            nc.scalar.copy(vh_aug[:, JT * VW:], ones_f32)

            bq_sb = persist.tile([P, FL // P], f32)
            bk_sb = persist.tile([P, FL // P], f32)
            nc.sync.dma_start(bq_sb, bq_d.rearrange("(a p) -> p a", p=P))
            nc.sync.dma_start(bk_sb, bk_d.rearrange("(a p) -> p a", p=P))
            bv_bc = persist.tile([P, FL], f32)
            nc.gpsimd.dma_start(bv_bc, bcast_ap(bv_d[:], P))
            bo_bc = persist.tile([P, DIM], f32)
            gamma_bc = persist.tile([P, DIM], f32)
            beta_bc = persist.tile([P, DIM], f32)
            eps_sb = persist.tile([P, 1], f32)
            nc.vector.memset(eps_sb, EPS)

            scope_stack = []

            def enter_scope(nm):
                while scope_stack:
                    n0, sid = scope_stack.pop()
                    nc.leave_named_scope(n0, sid, False)
                scope_stack.append((nm, nc.enter_named_scope(nm, False)[0]))

            # ---------------- q/k projections ----------------
            enter_scope("proj")
            for name, x_d, w_sb, b_sb, dstT in (
                ("q", xqT_d, wq_sb, bq_sb, qhT),
                ("k", xkT_d, wk_sb, bk_sb, khT),
            ):
                for tci in range(IC):
                    xt = xs.tile([P, DC, 512], bf16, tag="xqk",
                                 name=f"x{name}_{tci}")
                    for dc in range(DC):
                        nc.sync.dma_start(
                            xt[:, dc, :],
                            x_d[dc * P:(dc + 1) * P,
                                tci * 512:(tci + 1) * 512])
                    for fc in range(FL // P):
                        ps = avp.tile([P, 512], f32, tag="ps512",
                                      name=f"ps_{name}_{tci}_{fc}")
                        for dc in range(DC):
                            nc.tensor.matmul(
                                ps, w_sb[:, dc, fc * P:(fc + 1) * P],
                                xt[:, dc, :],
                                start=(dc == 0), stop=(dc == DC - 1))
                        for hf in range(2):
                            h = 2 * fc + hf
                            nc.scalar.activation(
                                dstT[:, h, tci * 512:(tci + 1) * 512],
                                ps[hf * HD:(hf + 1) * HD], IDENT,
                                bias=b_sb[hf * HD:(hf + 1) * HD, fc:fc + 1])

            # woT reuses the q/k weight slots now that they are free
            woA = ws.tile([P, 4, DIM], bf16, tag="wA", name="woA")
            woB = ws.tile([P, 4, DIM], bf16, tag="wB", name="woB")
            for a in range(4):
                nc.sync.dma_start(woA[:, a, :],
                                  woT_d[a * P:(a + 1) * P, :])
                nc.sync.dma_start(woB[:, a, :],
                                  woT_d[FL + a * P:FL + (a + 1) * P, :])

            # ---------------- v projection ----------------
            vh3 = vh_aug[:, :JT * VW].rearrange(
                "p (j h w) -> p j h w", h=NH, w=HW)
            bv3 = bv_bc.rearrange("p (h w) -> p h w", w=HD)
            for tci in range(IC):
                xvt = xs.tile([P, DC, 512], bf16, tag="xqk",
                              name=f"xv_{tci}")
                for dc in range(DC):
                    nc.sync.dma_start(
                        xvt[:, dc, :],
                        xvT_d[dc * P:(dc + 1) * P,
                              tci * 512:(tci + 1) * 512])
                for tj in range(4):
                    jt = tci * 4 + tj
                    ps = avp.tile([P, FL], f32, tag="ps512",
                                  name=f"ps_v_{jt}")
                    for dc in range(DC):
                        nc.tensor.matmul(
                            ps, xvt[:, dc, tj * P:(tj + 1) * P],
                            wv_sb[:, dc, :],
                            start=(dc == 0), stop=(dc == DC - 1))
                    nc.vector.scalar_tensor_tensor(
                        vh3[:, jt, :, :HD],
                        ps.rearrange("p (h w) -> p h w", w=HD), 0.0,
                        bv3, op0=ALU.add, op1=ALU.add)

            # ---------------- attention ----------------
            # half=1 (peer token half, cols TH:S) first so the pair exchange
            # overlaps the half=0 attention.
            cc_in = dram.tile([FL, TH], bf16)
            cc_sum = dram.tile([FL, TH], bf16)

            def attn_head(half, h):
                colb = half * TH
                pav = [avp.tile([P, 512], f32, tag="ps512",
                                name=f"pav_{half}_{h}_{i}") for i in range(2)]
                et_t = {}

                def S_step(jt):
                    psc = scp.tile([P, 1024], f32, tag="sc",
                                   name=f"sc_{half}_{h}_{jt}")
                    for i2 in range(2):
                        nc.tensor.matmul(
                            psc[:, i2 * 512:(i2 + 1) * 512],
                            khT[:, h, jt * P:(jt + 1) * P],
                            qhT[:, h, colb + i2 * 512:colb + (i2 + 1) * 512],
                            start=True, stop=True)
                    et = et_pool.tile([P, 1024], bf16, tag="et",
                                      name=f"et_{half}_{h}_{jt}")
                    nc.scalar.activation(et, psc, EXPF, scale=SCALE)
                    et_t[jt] = et

                def AV_step(jt):
                    et = et_t.pop(jt)
                    for i2 in range(2):
                        nc.tensor.matmul(
                            pav[i2],
                            vh_aug[:, jt * VW + h * HW:jt * VW + h * HW + P],
                            et[:, i2 * 512:(i2 + 1) * 512],
                            start=(jt == 0), stop=(jt == JT - 1))

                S_step(0)
                S_step(1)
                for jt in range(JT):
                    if jt + 2 < JT:
                        S_step(jt + 2)
                    AV_step(jt)

                # evict + normalize: row 64 holds the softmax denominator
                hb = (h % 2) * HD
                for i2 in range(2):
                    rr = bc_pool.tile([1, 2, 512], f32, tag="rr",
                                      name=f"rr_{half}_{h}_{i2}")
                    # custom-DVE recip can't read PSUM: stage row to SBUF
                    nc.vector.tensor_copy(rr[:, 1, :], pav[i2][HD:HD + 1])
                    nc.vector.reciprocal_approx_fast(rr[:, 0, :], rr[:, 1, :])
                    rb = bc_pool.tile([HD, 512], f32, tag="rb",
                                      name=f"rb_{half}_{h}_{i2}")
                    nc.gpsimd.partition_broadcast(rb, rr[:, 0, :])
                    nc.vector.scalar_tensor_tensor(
                        out_normT[hb:hb + HD, h // 2,
                                  colb + i2 * 512:colb + (i2 + 1) * 512],
                        pav[i2][:HD], 0.0, rb, op0=ALU.add, op1=ALU.mult)

            enter_scope("attnP")
            for h in range(NH):
                attn_head(1, h)

            # ---------------- pair exchange (fires under attnM) ----------
            enter_scope("exch")
            for pi in range(FL // P):
                nc.sync.dma_start(cc_in[pi * P:(pi + 1) * P, :],
                                  out_normT[:, pi, TH:S])
            nc.gpsimd.collective_compute(
                "AllReduce", ALU.add, replica_groups=PAIRS,
                ins=[cc_in.opt()], outs=[cc_sum.opt()])

            nc.gpsimd.dma_start(bo_bc, bcast_ap(bo_d[:], P))
            nc.gpsimd.dma_start(gamma_bc, bcast_ap(gamma_d[:], P))
            nc.gpsimd.dma_start(beta_bc, bcast_ap(beta_d[:], P))

            enter_scope("attnM2")
            peer_T = None
            for h in range(NH):
                attn_head(0, h)
                if h == 4:
                    # peer = (mine + peer) - mine; emitted this late so the
                    # collective is long done and the DVE never head-of-line
                    # blocks the eviction chain (a stall here re-throttles
                    # the PE clock for the rest of the kernel).
                    peer_T = ws.tile([P, FL // P, TH], bf16, tag="wC")
                    ta_all = xs.tile([P, FL // P, TH], bf16, tag="xqk",
                                     name="cc_stage")
                    nc.sync.dma_start(
                        ta_all,
                        cc_sum[:, :].rearrange("(a p) t -> p a t", p=P))
                    for pi in range(FL // P):
                        nc.vector.scalar_tensor_tensor(
                            peer_T[:, pi, :], ta_all[:, pi, :], 0.0,
                            out_normT[:, pi, TH:S],
                            op0=ALU.add, op1=ALU.subtract)

            # ---------------- output projection + layernorm ----------------
            enter_scope("final")
            for it in range(TH // P):
                psf = [avp.tile([P, 512], f32, tag="ps512",
                                name=f"psf_{it}_{f}") for f in range(2)]
                for fcc in range(2):
                    for cc in range(DC):
                        if cc < 4:
                            stat = out_normT[:, cc, it * P:(it + 1) * P]
                            mov = woA[:, cc, fcc * 512:(fcc + 1) * 512]
                        else:
                            stat = peer_T[:, cc - 4, it * P:(it + 1) * P]
                            mov = woB[:, cc - 4, fcc * 512:(fcc + 1) * 512]
                        nc.tensor.matmul(
                            psf[fcc], stat, mov,
                            start=(cc == 0), stop=(cc == DC - 1))
                xln = ln_pool.tile([P, DIM], f32, tag="xln", name=f"xln_{it}")
                for fcc in range(2):
                    nc.vector.scalar_tensor_tensor(
                        xln[:, fcc * 512:(fcc + 1) * 512], psf[fcc], 0.0,
                        bo_bc[:, fcc * 512:(fcc + 1) * 512],
                        op0=ALU.add, op1=ALU.add)
                stats = ln_pool.tile([P, 2, 6], f32, tag="st", name=f"st_{it}")
                for hf in range(2):
                    nc.vector.bn_stats(stats[:, hf, :],
                                       xln[:, hf * 512:(hf + 1) * 512])
                mv = ln_pool.tile([P, 2], f32, tag="mv", name=f"mv_{it}")
                nc.vector.bn_aggr(mv, stats)
                rstd = ln_pool.tile([P, 1], f32, tag="rstd", name=f"rstd_{it}")
                nc.scalar.activation(rstd, mv[:, 1:2], SQRTF, bias=eps_sb)
                nc.vector.reciprocal(rstd, rstd)
                nc.vector.scalar_tensor_tensor(
                    xln, xln, mv[:, 0:1], gamma_bc,
                    op0=ALU.subtract, op1=ALU.mult)
                nc.vector.scalar_tensor_tensor(
                    xln, xln, rstd, beta_bc, op0=ALU.mult, op1=ALU.add)
                nc.sync.dma_start(y_d[it * P:(it + 1) * P, :], xln)
            while scope_stack:
                n0, sid = scope_stack.pop()
                nc.leave_named_scope(n0, sid, False)

    nc.compile()
    return nc


def kernel(q, k, v, Wq, bq, Wk, bk, Wv, bv, Wo, bo, gamma, beta):
    from concourse.bass_utils import run_bass_kernel_spmd

    if "nc" not in _cache:
        _cache["nc"] = _build()
    nc = _cache["nc"]

    q = np.asarray(q, np.float32)
    k = np.asarray(k, np.float32)
    v = np.asarray(v, np.float32)
    Wq = np.asarray(Wq, np.float32); Wk = np.asarray(Wk, np.float32)
    Wv = np.asarray(Wv, np.float32); Wo = np.asarray(Wo, np.float32)
    bf = ml_dtypes.bfloat16
    WoT = Wo.T  # [c, f]
    in_maps = []
    for c in range(NCORES):
        b, g = c // 2, c % 2
        fsl = slice(g * FL, (g + 1) * FL)
        # my token half first (so "my half" is always columns 0:TH)
        xq = q[b].T if g == 0 else np.concatenate(
            [q[b].T[:, TH:], q[b].T[:, :TH]], axis=1)
        # Wo rows rotated: [my features, peer features]
        woT = np.concatenate([WoT[g * FL:(g + 1) * FL],
                              WoT[(1 - g) * FL:(2 - g) * FL]], axis=0)
        in_maps.append({
            "xqT": np.ascontiguousarray(xq).astype(bf),
            "xkT": np.ascontiguousarray(k[b].T).astype(bf),
            "xvT": np.ascontiguousarray(v[b].T).astype(bf),
            "wqT": np.ascontiguousarray(Wq[fsl, :].T).astype(bf),
            "wkT": np.ascontiguousarray(Wk[fsl, :].T).astype(bf),
            "wvT": np.ascontiguousarray(Wv[fsl, :].T).astype(bf),
            "woT": np.ascontiguousarray(woT).astype(bf),
            "bq": np.asarray(bq, np.float32)[fsl],
            "bk": np.asarray(bk, np.float32)[fsl],
            "bv": np.asarray(bv, np.float32)[fsl],
            "bo": np.asarray(bo, np.float32),
            "gamma": np.asarray(gamma, np.float32),
            "beta": np.asarray(beta, np.float32),
        })
    res = run_bass_kernel_spmd(nc, in_maps, list(range(NCORES)),
                               trace=_cache.get("trace", False))
    _cache["last_res"] = res
    y = np.empty((B, S, DIM), np.float32)
    for c in range(NCORES):
        b, g = c // 2, c % 2
        y[b, g * TH:(g + 1) * TH, :] = res.results[c]["y"]
    return y
